# revision 17
# baseline (speedup 1.0000x reference)
"""Trainium2 Bass kernel: batched causal single-head self-attention.

Reference computation (per batch b):
    q = x @ Wq; k = x @ Wk; v = x @ Wv          # [T, H] each, contraction over E
    S = (q @ k^T) / sqrt(H)                     # [T, T]
    P = softmax(causal_mask(S), axis=-1)
    out = P @ v                                 # [T, H]

Shapes: x [512, 256, 384] f32, W* [384, 64] f32, out [512, 256, 64] f32.
Sharding: pure data parallel, 64 batches per NeuronCore across 8 cores.

Device algorithm per batch (matmul operands bf16, fp32 PSUM accumulation):
  - host ships xt = x^T per batch ([E, T] layout, E on partitions, p-major
    DRAM so every DMA is one contiguous run per partition).
  - [q^T; k^T] = [Wq|Wk]^T @ xt     (one packed 128-wide stationary, 3
    E-chunks, both batches of a pair as one N=512 moving operand)
  - v        = xt_chunk.T @ Wv      (xt chunks [e,t] as stationary -> v in
    [t, h] layout directly; no transpose anywhere)
  - S^T      = k^T.T @ q^T          ([tk, tq]; lower-left T/4 block skipped)
  - P        = exp(0.125 * S^T)     (ScalarE; no max-subtraction, |s|<~45)
  - P       *= causal 0/1 mask      (GpSimd, only the two diagonal blocks)
  - out_aug[tq, 0:65] = sum_tk P[tk,tq] * [v|1][tk]  (col 64 = softmax
    denominator via the ones column); divide on DVE, store bf16.

The emission is software-pipelined at pair granularity: step g issues
PROJ(g), SCORES(g-4), OUT(g-6) so the in-order per-engine streams never
block on the exp->mask->out dependency chain, and a warm-up burst of
matmuls keeps the PE HAM clock-gate open while the first input DMA lands.
"""

import numpy as np
import ml_dtypes

B, T, E, H = 512, 256, 384, 64
NCORES = 8
BPC = B // NCORES  # 64
P = 128
EC = E // P  # 3
HP1 = H + 1  # 65

_cache: dict = {}


def _install_ntff_hook():
    """Shim antenv.axon_hooks (absent in this image) so run_bass_kernel_spmd
    trace=True can capture NTFF profiles via the axon .so's C ABI."""
    import contextlib
    import ctypes
    import sys
    import types

    if "antenv.axon_hooks" in sys.modules:
        return
    so_path = "/opt/axon/libaxon_pjrt.so"
    lib = ctypes.CDLL(so_path)
    if not hasattr(lib, "axon_start_nrt_profile"):
        return
    lib.axon_start_nrt_profile.argtypes = [
        ctypes.POINTER(ctypes.c_int64),
        ctypes.c_size_t,
    ]
    lib.axon_start_nrt_profile.restype = ctypes.c_int64
    lib.axon_stop_nrt_profile.argtypes = [ctypes.c_char_p]
    lib.axon_stop_nrt_profile.restype = ctypes.c_int64

    @contextlib.contextmanager
    def _hook(output_dir, device_ids):
        import jax

        jax.devices()
        if device_ids:
            ids = (ctypes.c_int64 * len(device_ids))(*device_ids)
            rc = lib.axon_start_nrt_profile(ids, len(device_ids))
        else:
            rc = lib.axon_start_nrt_profile(None, 0)
        if rc != 0:
            raise RuntimeError(f"axon_start_nrt_profile rc={rc}")
        try:
            yield
        finally:
            n = lib.axon_stop_nrt_profile(str(output_dir).encode())
            if n < 0:
                raise RuntimeError(f"axon_stop_nrt_profile rc={n}")
            print(f"profile: {n} file(s) written to {output_dir}", file=sys.stderr)

    mod = types.ModuleType("antenv.axon_hooks")
    _state = {"hook": _hook}
    mod.get_axon_ntff_profile_hook = lambda: _state["hook"]
    mod.set_axon_ntff_profile_hook = lambda h: _state.__setitem__("hook", h)
    sys.modules["antenv.axon_hooks"] = mod


def _build_program(bpc):
    import concourse.bacc as bacc
    import concourse.mybir as mybir
    import concourse.tile as tile

    f32 = mybir.dt.float32
    bf16 = mybir.dt.bfloat16
    Exp = mybir.ActivationFunctionType.Exp
    Mult = mybir.AluOpType.mult

    nc = bacc.Bacc(
        "TRN2",
        target_bir_lowering=False,
        debug=False,
        enable_asserts=False,
        num_devices=NCORES,
    )
    # p-major DRAM layouts: one contiguous run per partition per DMA.
    xt_d = nc.dram_tensor("xt", [P, bpc, EC, T], bf16, kind="ExternalInput").ap()
    wqk_d = nc.dram_tensor("wqk", [P, EC, P], bf16, kind="ExternalInput").ap()
    wv_d = nc.dram_tensor("wv", [P, EC, H], bf16, kind="ExternalInput").ap()
    tril_d = nc.dram_tensor("tril", [P, P], bf16, kind="ExternalInput").ap()
    out_d = nc.dram_tensor("out", [P, bpc, 2, H], bf16, kind="ExternalOutput").ap()

    OC = 8  # batches per octet (DMA granularity)
    PPO = OC // 2  # pairs per octet
    assert bpc % OC == 0
    nocts = bpc // OC
    npairs = bpc // 2
    SC_LAG = 5  # SCORES(g) at step g+SC_LAG (octet projections + shift done)
    OUT_LAG = 7  # OUT(g) at step g+OUT_LAG (exp+mask latency hidden)
    NVAUG = 10

    with tile.TileContext(nc) as tc:
        with (
            tc.tile_pool(name="const", bufs=1) as constp,
            tc.tile_pool(name="xin", bufs=3) as xpool,
            tc.tile_pool(name="qksb", bufs=4) as qkpool,
            tc.tile_pool(name="psb", bufs=6) as ppool,
            tc.tile_pool(name="osb", bufs=3) as opool,
            tc.tile_pool(name="rec", bufs=3) as rpool,
            tc.tile_pool(name="ps_qk", bufs=2, space="PSUM") as ps_qk,
            tc.tile_pool(name="ps_v", bufs=2, space="PSUM") as ps_v,
            tc.tile_pool(name="ps_s", bufs=2, space="PSUM") as ps_s,
            tc.tile_pool(name="ps_o", bufs=2, space="PSUM") as ps_o,
        ):
            wqk = constp.tile([P, EC, P], bf16)
            nc.sync.dma_start(wqk, wqk_d)
            wv = constp.tile([P, EC, H], bf16)
            nc.sync.dma_start(wv, wv_d)
            tril = constp.tile([P, P], bf16)
            nc.sync.dma_start(tril, tril_d)
            trilb = tril[:, None, :].to_broadcast([P, 2, P])

            # k^T staging padded to 128 partitions with zero rows 64:128 so
            # the scores matmuls use full-width stationaries; shift-DMA fills
            # rows 0:64 each octet, the zero rows persist.
            kabs = []
            for i in range(3):
                kt = constp.tile([P, OC, T], bf16, name=f"kab{i}")
                nc.gpsimd.memset(kt[H:P], 0.0)
                kabs.append(kt)
            # v staging [tk, h] with a persistent ones column at h=64
            vaugs = []
            for i in range(NVAUG):
                vt = constp.tile([P, 2, 2, HP1], bf16, name=f"vaug{i}")
                nc.gpsimd.memset(vt[:, :, :, H : H + 1], 1.0)
                vaugs.append(vt)

            # HAM warm-up: keep the PE busy (~5us) while the first x octet
            # streams in, so real matmuls start at the 2.4 GHz clock. The
            # zeroed scratch has no DRAM dependency, so this starts as soon
            # as the engines come up.
            wz = constp.tile([P, 3 * P], bf16, name="wz")
            nc.vector.memset(wz, 0.0)
            warm = ps_s.tile([P, 3 * P], f32, name="s_ps")
            for i in range(20):
                nc.tensor.matmul(
                    warm,
                    wz[:, 0:P],
                    wz,
                    start=True,
                    stop=True,
                )

            xts: dict = {}
            qks: dict = {}
            osbs: dict = {}
            psbs: dict = {}

            def ensure_oct(o):
                if o >= nocts or o in xts:
                    return
                b0 = OC * o
                xt = xpool.tile([P, OC, EC, T], bf16)
                if o == 0:
                    # pair-sized chunks so the first projections start ASAP
                    for i in range(OC // 2):
                        nc.sync.dma_start(
                            xt[:, 2 * i : 2 * i + 2],
                            xt_d[:, b0 + 2 * i : b0 + 2 * i + 2],
                        )
                else:
                    nc.sync.dma_start(
                        xt[:, 0 : OC // 2], xt_d[:, b0 : b0 + OC // 2]
                    )
                    nc.sync.dma_start(
                        xt[:, OC // 2 : OC], xt_d[:, b0 + OC // 2 : b0 + OC]
                    )
                xts[o] = xt
                qks[o] = qkpool.tile([P, OC, T], bf16, name="qk_sb")

            def proj_mms(g):
                o, pr = divmod(g, PPO)
                s0 = 2 * pr
                xt = xts[o]
                qk_ps = ps_qk.tile([P, 2, T], f32)
                v_psf = ps_v.tile([P, 2, 2, P], f32)
                v_ps = v_psf[:, :, :, 0:H]
                qk_ops = [
                    (lambda c=c: nc.tensor.matmul(
                        qk_ps,
                        wqk[:, c, :],
                        xt[:, s0 : s0 + 2, c, :],
                        start=(c == 0),
                        stop=(c == EC - 1),
                    ))
                    for c in range(EC)
                ]
                v_ops = [
                    (lambda s=s, j=j, c=c: nc.tensor.matmul(
                        v_ps[:, s, j, :],
                        xt[:, s0 + s, c, j * P : (j + 1) * P],
                        wv[:, c, :],
                        start=(c == 0),
                        stop=(c == EC - 1),
                    ))
                    for s in range(2)
                    for j in range(2)
                    for c in range(EC)
                ]
                return qk_ps, v_ps, qk_ops, v_ops

            def proj_copies(g, qk_ps, v_ps):
                o, pr = divmod(g, PPO)
                s0 = 2 * pr
                qk_sb = qks[o]
                if pr % 2 == 0:
                    nc.scalar.copy(qk_sb[:, s0 : s0 + 2, :], qk_ps)
                else:
                    nc.vector.tensor_copy(qk_sb[:, s0 : s0 + 2, :], qk_ps)
                nc.vector.tensor_copy(vaugs[g % NVAUG][:, :, :, 0:H], v_ps)

            def scores_mms(g):
                o, pr = divmod(g, PPO)
                s0 = 2 * pr
                qk_sb, k_sb = qks[o], kabs[o % 3]
                p_sb = ppool.tile([P, 2, 3 * P], bf16, name="p_sb")
                psbs[g] = p_sb
                s_pss = []
                ops = []
                for s in range(2):
                    s_ps = ps_s.tile([P, 3 * P], f32, name="s_ps")
                    s_pss.append(s_ps)
                    ops.append(lambda s=s, s_ps=s_ps: nc.tensor.matmul(
                        s_ps[:, 0:T],
                        k_sb[:, s0 + s, 0:P],
                        qk_sb[:, s0 + s, :],
                        start=True,
                        stop=True,
                    ))
                    ops.append(lambda s=s, s_ps=s_ps: nc.tensor.matmul(
                        s_ps[:, T : 3 * P],
                        k_sb[:, s0 + s, P:T],
                        qk_sb[:, s0 + s, P:T],
                        start=True,
                        stop=True,
                    ))
                return p_sb, s_pss, ops

            def scores_post(g, p_sb, s_pss, s):
                nc.scalar.activation(p_sb[:, s, :], s_pss[s], Exp, scale=0.125)

            def scores_mask(g, p_sb):
                # multiplicative causal mask on the two diagonal blocks
                nc.gpsimd.tensor_tensor(
                    p_sb[:, :, 0:P], p_sb[:, :, 0:P], trilb, Mult
                )
                nc.gpsimd.tensor_tensor(
                    p_sb[:, :, T : 3 * P], p_sb[:, :, T : 3 * P], trilb, Mult
                )

            def out_mms(g):
                o, pr = divmod(g, PPO)
                if pr == 0 and o not in osbs:
                    osbs[o] = opool.tile([P, OC, 2, H], bf16, name="o_sb")
                v_aug = vaugs[g % NVAUG]
                p_sb = psbs.pop(g)
                o_ps = ps_o.tile([P, 2, 2, HP1], f32)
                ops = []
                for s in range(2):
                    ops.append(lambda s=s: nc.tensor.matmul(
                        o_ps[:, s, 0, :],
                        p_sb[:, s, 0:P],
                        v_aug[:, s, 0, :],
                        start=True,
                        stop=True,
                    ))
                    ops.append(lambda s=s: nc.tensor.matmul(
                        o_ps[:, s, 1, :],
                        p_sb[:, s, P:T],
                        v_aug[:, s, 0, :],
                        start=True,
                        stop=False,
                    ))
                    ops.append(lambda s=s: nc.tensor.matmul(
                        o_ps[:, s, 1, :],
                        p_sb[:, s, T : 3 * P],
                        v_aug[:, s, 1, :],
                        start=False,
                        stop=True,
                    ))
                return o_ps, ops

            def out_post(g, o_ps):
                o, pr = divmod(g, PPO)
                s0 = 2 * pr
                rec = rpool.tile([P, 2, 2, 1], f32)
                nc.vector.reciprocal(rec, o_ps[:, :, :, H : H + 1])
                nc.vector.tensor_tensor(
                    osbs[o][:, s0 : s0 + 2, :, :],
                    o_ps[:, :, :, 0:H],
                    rec.to_broadcast([P, 2, 2, H]),
                    Mult,
                )

            def interleave(long_ops, short_ops):
                # alternate long (N>=256) and short matmuls so short-op
                # LDWEIGHTS hide under the long moving streams
                out = []
                li, si = 0, 0
                while li < len(long_ops) or si < len(short_ops):
                    if li < len(long_ops):
                        out.append(long_ops[li]); li += 1
                    if si < len(short_ops):
                        out.append(short_ops[si]); si += 1
                return out

            ensure_oct(0)
            for step in range(npairs + OUT_LAG):
                g = step if step < npairs else None
                gs = step - SC_LAG if 0 <= step - SC_LAG < npairs else None
                go = step - OUT_LAG if 0 <= step - OUT_LAG < npairs else None

                if g is not None and g % PPO == 0:
                    ensure_oct(g // PPO + 1)

                qk_ops, v_ops, sc_ops, o_ops = [], [], [], []
                if g is not None:
                    qk_ps, v_ps, qk_ops, v_ops = proj_mms(g)
                if gs is not None:
                    p_sb, s_pss, sc_ops = scores_mms(gs)
                if go is not None:
                    o_ps, o_ops = out_mms(go)

                # PE stream: long qk/scores MMs alternated with short out MMs,
                # then the LDW-bound v section
                for op in interleave(qk_ops + sc_ops, o_ops):
                    op()
                if gs is not None:
                    scores_post(gs, p_sb, s_pss, 0)
                    scores_post(gs, p_sb, s_pss, 1)
                for op in v_ops:
                    op()

                if go is not None:
                    out_post(go, o_ps)
                if g is not None:
                    proj_copies(g, qk_ps, v_ps)
                if gs is not None:
                    scores_mask(gs, p_sb)

                # half-octet k-shift as soon as 2 pairs of qk are staged
                if g is not None and g % 2 == 1:
                    o, pr = divmod(g, PPO)
                    h0 = (pr - 1) * 2
                    nc.sync.dma_start(
                        kabs[o % 3][0:H, h0 : h0 + 4], qks[o][H:P, h0 : h0 + 4]
                    )
                # half-octet out store; per pair for the final octet so the
                # last DMA is small and completion staggers
                if go is not None:
                    o, pr = divmod(go, PPO)
                    if o == nocts - 1:
                        nc.sync.dma_start(
                            out_d[:, OC * o + 2 * pr : OC * o + 2 * pr + 2],
                            osbs[o][:, 2 * pr : 2 * pr + 2],
                        )
                        if pr == PPO - 1:
                            osbs.pop(o)
                    elif go % 2 == 1:
                        h0 = (pr - 1) * 2
                        nc.sync.dma_start(
                            out_d[:, OC * o + h0 : OC * o + h0 + 4],
                            osbs[o][:, h0 : h0 + 4],
                        )
                        if pr == PPO - 1:
                            osbs.pop(o)

    nc.compile()
    return nc


def _prep_inputs(x, Wq, Wk, Wv, bpc):
    bf = ml_dtypes.bfloat16
    nb = NCORES * bpc
    x = np.asarray(x, dtype=np.float32)[:nb]
    # [b, t, e] -> [p, b, c, t] with e = c*128 + p  (p-major for the DMA)
    xt = np.ascontiguousarray(
        x.reshape(nb, T, EC, P).transpose(3, 0, 2, 1)
    ).astype(bf)
    wqk = np.concatenate(
        [np.asarray(Wq, np.float32), np.asarray(Wk, np.float32)], axis=1
    )  # [E, 128]
    wqk = np.ascontiguousarray(wqk.reshape(EC, P, P).transpose(1, 0, 2)).astype(bf)
    wv = np.ascontiguousarray(
        np.asarray(Wv, np.float32).reshape(EC, P, H).transpose(1, 0, 2)
    ).astype(bf)
    tril01 = (np.arange(P)[:, None] <= np.arange(P)[None, :]).astype(np.float32)
    tril = tril01.astype(bf)
    per_core = []
    for c in range(NCORES):
        per_core.append(
            {
                "xt": np.ascontiguousarray(xt[:, c * bpc : (c + 1) * bpc]),
                "wqk": wqk,
                "wv": wv,
                "tril": tril,
            }
        )
    return per_core


def kernel(x, Wq, Wk, Wv, _trace=False, _bpc=BPC):
    """Full inputs in, full output out. Shards batch dim over 8 NeuronCores."""
    from concourse import bass_utils

    if _trace:
        _install_ntff_hook()

    key = ("prog", _bpc)
    if key not in _cache:
        _cache[key] = _build_program(_bpc)
    nc = _cache[key]

    in_maps = _prep_inputs(x, Wq, Wk, Wv, _bpc)
    res = bass_utils.run_bass_kernel_spmd(
        nc, in_maps, core_ids=list(range(NCORES)), trace=_trace
    )
    _cache["last_result"] = res
    outs = []
    for r in res.results:
        o = np.asarray(r["out"])  # [P, bpc, 2, H] bf16
        outs.append(
            o.transpose(1, 2, 0, 3).reshape(_bpc, T, H).astype(np.float32)
        )
    return np.concatenate(outs, axis=0)
